# revision 19
# baseline (speedup 1.0000x reference)
"""Multi-head attention Bass kernel for Trainium2 (8 NeuronCores).

Problem: B=2, N=4096, E=768, H=12 heads of dim 64 (nn_MultiHeadAttention).
Sharding: 2 batches x 4 head-groups (3 heads each) = 8 cores.

Per-core pipeline (fp8 DoubleRow edition):
  - QKV projection in bf16 (x and w_qkv cast to bf16 on host; 1 cyc/row).
  - Q stored as fp8e4 (hi, lo) pair: hi = e4m3(psum), lo = e4m3(psum+bq-hi),
    so hi+lo = q+bq to ~fp8^2 precision and the Q bias rides in lo.
  - K stored as fp8e4 duplicated across the two DoubleRow k-subtiles.
  - Scores: one fp8 DoubleRow matmul per kv tile: (K,K) x (Qhi,Qlo) ->
    s = (q+bq).K at 0.5 cyc/row (2x f32r, with Q at ~2^-8 precision).
  - exp: ACT Exp(scale=1/8) psum->fp8 directly, or (route split) DVE copy to
    SBUF + Pool (gpsimd) pow((e^1/8)^s) -> fp8; both bit-exact RNE.
  - PV: one fp8 DoubleRow matmul per kv PAIR (2x128 contraction):
    lhsT = V tile [128, 2, 96] (64 v-dims + ones col + 31 pad), rhs = p pair
    tile. 4x f32r throughput. Softmax denominator from the ones column.
  - Output projection in f32r against w_proj rows -> partial [N, 768].
Host: sums the 4 partials per batch and adds the (bias-folded) b_proj.

PSUM budget (8 banks): "sc" ring = 3 x 2-bank tiles shared by stage-A
psq/psv and the per-kv-pair score tiles; "pv" ring = 2 x 1-bank tiles shared
by the PV accumulators (head-major: ~one live at a time) and out-proj tiles.

Bias handling (exact algebra): K bias drops out of softmax; V bias commutes
through normalization and folds into b_proj (host); Q bias folded into Q-lo.
"""

import sys

sys.path.insert(0, "/opt/trn_rl_repo")

import numpy as np
import ml_dtypes

import concourse.bass as bass  # noqa: E402
import concourse.mybir as mybir  # noqa: E402
import concourse.tile as tile  # noqa: E402
from concourse import bacc  # noqa: E402
from concourse.bass_utils import run_bass_kernel_spmd  # noqa: E402

F32 = mybir.dt.float32
F32R = mybir.dt.float32r
BF16 = mybir.dt.bfloat16
FP8 = mybir.dt.float8e4
AF = mybir.ActivationFunctionType
ALU = mybir.AluOpType
DRMODE = mybir.MatmulPerfMode.DoubleRow

B, N, E = 2, 4096, 768
H, HD = 12, 64
NH = 3          # heads per core
M_GROUPS = 4    # head groups (tensor parallel)
VW = 96         # PV lhsT width: 64 v-dims + 1 ones + 31 pad (mult of 32)

# tuning knobs (TimelineSim-swept)
KVQ = 2         # kv tiles per score-psum tile; sc tile = [128,KVQ,512]
SC_BUFS = 3     # score-ring depth (KVQ/2 banks each)
POOL_PAT = (2, 5, 7)  # which (idx % 8) tiles take the DVE-copy + Pool pow route
AHEAD = 2       # score quads emitted ahead of the consuming exp
PVD = 4         # PV consumption delayed this many jobs behind exp emission
OST_ACT_EVERY = 4  # every Nth out-proj copy on ACT (0 = all DVE)


def build_nc(n_tokens=N, num_devices=8):
    """Build the per-core Bass module (SPMD: same program, different data)."""
    n = n_tokens
    NQG = n // 512          # q groups of 512
    NKV = n // 128          # kv blocks of 128
    KE = E // 128           # contraction tiles over E

    nc = bacc.Bacc("TRN2", target_bir_lowering=False, debug=False,
                   num_devices=num_devices)

    xT = nc.dram_tensor("xT", [E, n], BF16, kind="ExternalInput")
    wqkT = nc.dram_tensor("wqkT", [E, 3 * 128], BF16, kind="ExternalInput")
    wvT = nc.dram_tensor("wvT", [E, NH * HD], BF16, kind="ExternalInput")
    bq = nc.dram_tensor("bq", [2, 128], F32, kind="ExternalInput")
    wpT = nc.dram_tensor("wpT", [HD, NH, E], F32R, kind="ExternalInput")
    cpow = nc.dram_tensor("cpow", [128, 1], F32, kind="ExternalInput")
    out = nc.dram_tensor("out", [n, E], F32, kind="ExternalOutput")

    with tile.TileContext(nc) as tc:
        with (
            tc.tile_pool(name="perm", bufs=1) as perm,
            tc.tile_pool(name="wpool", bufs=1) as wpool,
        ):
            # Persistent SBUF tensors.
            # Q/K for heads 0 (partitions 0:64) and 1 (64:128); middle dim:
            # q_sb = (hi, lo), k_sb = duplicated K for the DR subtile pair.
            q_sb = perm.tile([128, 2, n], FP8, name="q_sb")
            k_sb = perm.tile([128, 2, n], FP8, name="k_sb")
            # head 2 on partitions 0:64
            q2_sb = perm.tile([64, 2, n], FP8, name="q2_sb")
            k2_sb = perm.tile([64, 2, n], FP8, name="k2_sb")
            # V in [kv, d] layout: per kv-block of 128 tokens,
            # 3 heads x (64 dims + ones + 31 pad).
            v_sb = perm.tile([128, NKV, NH, VW], FP8, name="v_sb")

            wqkT_sb = wpool.tile([128, KE, 3 * 128], BF16, name="wqk_sb")
            wvT_sb = wpool.tile([128, KE, NH * HD], BF16, name="wv_sb")
            wpT_sb = wpool.tile([64, NH, E], F32R, name="wp_sb")
            bq_sb = wpool.tile([128, 2], F32, name="bq_sb")
            cp_sb = wpool.tile([128, 1], F32, name="cp_sb")

            nc.sync.dma_start(wqkT_sb[:], wqkT.rearrange("(a p) c -> p a c", p=128))
            nc.sync.dma_start(wvT_sb[:], wvT.rearrange("(a p) c -> p a c", p=128))
            nc.sync.dma_start(wpT_sb[:], wpT[:])
            nc.sync.dma_start(bq_sb[:], bq.rearrange("a p -> p a"))
            nc.sync.dma_start(cp_sb[:], cpow[:])

            # ones column for the softmax-denominator trick (pad cols 65:96
            # only feed psum partitions 65:96, which are never read).
            nc.vector.memset(v_sb[:, :, :, HD:HD + 1], 1.0)

            with (
                tc.tile_pool(name="scpsum", bufs=1, space="PSUM") as scpsum,
                tc.tile_pool(name="pvpsum", bufs=1, space="PSUM") as pvpsum,
                tc.tile_pool(name="xpool", bufs=16) as xpool,
                tc.tile_pool(name="spool", bufs=3) as spool,
            ):
                # ---- Stage A: QKV projection (bf16), quantize to fp8 ----
                # psq/psv tiles ride the "sc" ring (shared with score tiles).
                def emit_stageA(ng):
                    xts = []
                    for k in range(KE):
                        xt = xpool.tile([128, 512], BF16, tag="xt",
                                        name=f"xt{ng}_{k}")
                        nc.sync.dma_start(xt[:], xT[k * 128:(k + 1) * 128,
                                                    ng * 512:(ng + 1) * 512])
                        xts.append(xt)
                    qs = slice(ng * 512, (ng + 1) * 512)
                    for m in range(3):
                        psq = scpsum.tile([128, 512], F32, tag="sc",
                                          bufs=SC_BUFS, name=f"psq{ng}_{m}")
                        for k in range(KE):
                            nc.tensor.matmul(psq[:],
                                             wqkT_sb[:, k, m * 128:(m + 1) * 128],
                                             xts[k][:], start=(k == 0),
                                             stop=(k == KE - 1))
                        if m == 0:      # Q heads 0,1
                            nc.scalar.copy(q_sb[:, 0, qs], psq[:])
                            nc.vector.scalar_tensor_tensor(
                                q_sb[:, 1, qs], psq[:], bq_sb[:, 0:1],
                                q_sb[:, 0, qs], op0=ALU.add, op1=ALU.subtract)
                        elif m == 1:    # K heads 0,1 (+ dup on Pool)
                            nc.scalar.copy(k_sb[:, 0, qs], psq[:])
                            nc.sync.dma_start(k_sb[:, 1, qs], k_sb[:, 0, qs])
                        else:           # m2 = [Q2 ; K2]
                            nc.scalar.copy(q2_sb[:, 0, qs], psq[0:64, :])
                            nc.vector.scalar_tensor_tensor(
                                q2_sb[:, 1, qs], psq[0:64, :], bq_sb[0:64, 1:2],
                                q2_sb[:, 0, qs], op0=ALU.add, op1=ALU.subtract)
                            # K2 must live on partitions 0:64 (same as Q2).
                            # DMA can't read PSUM: stage fp8 in SBUF, then a
                            # partition-shifting SBUF->SBUF DMA, then Pool dup.
                            k2st = xpool.tile([128, 512], FP8, tag="k2st",
                                              bufs=2, name=f"k2st{ng}")
                            nc.vector.tensor_copy(k2st[64:128, :],
                                                  psq[64:128, :])
                            nc.sync.dma_start(k2_sb[:, 0, qs], k2st[64:128, :])
                            nc.sync.dma_start(k2_sb[:, 1, qs], k2st[64:128, :])
                    # V projection (bf16): 2 kv-blocks per psum tile
                    for vj in range(2):
                        psv = scpsum.tile([128, 2, NH * HD], F32, tag="sc",
                                          bufs=SC_BUFS, name=f"psv{ng}_{vj}")
                        for j in range(2):
                            jj = 2 * vj + j
                            for k in range(KE):
                                nc.tensor.matmul(
                                    psv[:, j, :],
                                    xts[k][:, jj * 128:(jj + 1) * 128],
                                    wvT_sb[:, k, :], start=(k == 0),
                                    stop=(k == KE - 1))
                        kvt = ng * 4 + 2 * vj
                        nc.scalar.copy(
                            v_sb[:, kvt:kvt + 2, :, 0:HD],
                            psv.rearrange("p a (h c) -> p a h c", c=HD))

                # ---- Stage B+C: software-pipelined attention (head-major) --
                # h -> (q tile, k tile, partition base)
                HEADS = {0: (q_sb, k_sb, 0), 1: (q_sb, k_sb, 64),
                         2: (q2_sb, k2_sb, 0)}
                NQD = NKV // KVQ   # score quads per (qg, h)
                jobs = [(0, h, kvq) for kvq in range(NQD) for h in (0, 1)]
                jobs += [(0, 2, kvq) for kvq in range(NQD)]
                jobs += [(qg, h, kvq) for qg in range(1, NQG)
                         for h in range(NH) for kvq in range(NQD)]
                pvp_tiles = {}
                yn = {}

                def emit_scores(qg, h, kvq):
                    qsl = slice(qg * 512, (qg + 1) * 512)
                    qt, kt, pb = HEADS[h]
                    sc = scpsum.tile([128, KVQ, 512], F32, tag="sc",
                                     bufs=SC_BUFS, name=f"sc{qg}_{h}_{kvq}")
                    for j in range(KVQ):
                        kv = kvq * KVQ + j
                        if h == 2:
                            lhs = kt[:, :, kv * 128:(kv + 1) * 128]
                            rhs = qt[:, :, qsl]
                        else:
                            lhs = kt[pb:pb + 64, :, kv * 128:(kv + 1) * 128]
                            rhs = qt[pb:pb + 64, :, qsl]
                        nc.tensor.matmul(sc[:, j, :], lhs, rhs, start=True,
                                         stop=True, perf_mode=DRMODE)
                    return sc

                def emit_norm(qg, h):
                    pv = pvp_tiles[(qg, h)]
                    r = spool.tile([1, 512], F32, tag="r", bufs=2,
                                   name=f"r{qg}_{h}")
                    nc.vector.reciprocal(r[:], pv[HD:HD + 1, :])
                    rb = spool.tile([64, 512], F32, tag="rb", bufs=2,
                                    name=f"rb{qg}_{h}")
                    nc.gpsimd.partition_broadcast(rb[:], r[:])
                    ynt = spool.tile([64, 512], F32R, tag="yn", bufs=6,
                                     name=f"yn{qg}_{h}")
                    nc.vector.tensor_mul(ynt[:], pv[0:HD, :], rb[:])
                    yn[(qg, h)] = ynt

                def proj_thunks(qg):
                    thunks = []
                    ost_i = 0
                    for f in range(2):
                        fw = 512 if f == 0 else E - 512
                        fsl = slice(f * 512, f * 512 + fw)
                        for qb in range(4):
                            ost_i += 1
                            def blk(qg=qg, f=f, qb=qb, fw=fw, fsl=fsl,
                                    on_act=(OST_ACT_EVERY > 0
                                            and ost_i % OST_ACT_EVERY == 0)):
                                pp = pvpsum.tile([128, fw], F32, tag="pv",
                                                 bufs=2, name=f"pp{qg}_{f}_{qb}")
                                for h in range(NH):
                                    nc.tensor.matmul(
                                        pp[:],
                                        yn[(qg, h)][:, qb * 128:(qb + 1) * 128],
                                        wpT_sb[:, h, fsl],
                                        start=(h == 0), stop=(h == NH - 1))
                                ost = spool.tile([128, fw], F32, tag="ost",
                                                 bufs=4, name=f"ost{qg}_{f}_{qb}")
                                if on_act:
                                    nc.scalar.copy(ost[:], pp[:])
                                else:
                                    nc.vector.tensor_copy(ost[:], pp[:])
                                nc.sync.dma_start(
                                    out[qg * 512 + qb * 128:
                                        qg * 512 + (qb + 1) * 128, fsl], ost[:])
                            thunks.append(blk)
                    return thunks

                laggard = []   # delayed PV thunks: (emit_at_idx, fn)

                def flush_laggards(now):
                    while laggard and laggard[0][0] <= now:
                        laggard.pop(0)[1]()

                def do_job(idx, qg, hh, kvq, sc):
                    if kvq == 0:
                        pvp_tiles[(qg, hh)] = pvpsum.tile(
                            [VW, 512], F32, tag="pv", bufs=2,
                            name=f"pv{qg}_{hh}")
                    p_tile = spool.tile([128, KVQ, 512], FP8, tag="p",
                                        bufs=PVD + 3, name=f"p{qg}_{hh}_{kvq}")
                    if idx % 8 in POOL_PAT:
                        # Pool route: DVE copies scores to SBUF, gpsimd pow
                        scs = spool.tile([128, KVQ, 512], F32, tag="scs",
                                         bufs=3, name=f"scs{qg}_{hh}_{kvq}")
                        nc.vector.tensor_copy(scs[:], sc[:])
                        nc.gpsimd.tensor_tensor(
                            p_tile[:],
                            cp_sb[:, 0:1].broadcast_to([128, KVQ, 512]),
                            scs[:], op=ALU.pow)
                    else:
                        nc.scalar.activation(p_tile[:], sc[:], AF.Exp,
                                             scale=0.125)

                    def mk_pv(qg=qg, hh=hh, kvq=kvq, p_tile=p_tile):
                        def pv():
                            for t in range(KVQ // 2):
                                kv = kvq * KVQ + 2 * t
                                nc.tensor.matmul(
                                    pvp_tiles[(qg, hh)],
                                    v_sb[:, kv:kv + 2, hh, :],
                                    p_tile[:, 2 * t:2 * t + 2, :],
                                    start=(kv == 0), stop=(kv == NKV - 2),
                                    perf_mode=DRMODE)
                            if kvq == NQD - 1:
                                emit_norm(qg, hh)
                                if hh == 2:
                                    for di, blk in enumerate(proj_thunks(qg)):
                                        laggard.append((idx + PVD + 1 + di, blk))
                        return pv

                    laggard.append((idx + PVD, mk_pv()))
                    flush_laggards(idx)

                # Interleaved prologue: after stage-A chunk ng, run the two
                # first-head jobs (qg0, h0, kvq = 2ng, 2ng+1) whose K/V were
                # just produced. Keeps ACT/Pool fed during the projections.
                emitted = 0
                for ng in range(NQG):
                    emit_stageA(ng)
                    for _ in range(2 * NQD // NQG):
                        qg, hh, kvq = jobs[emitted]
                        sc = emit_scores(qg, hh, kvq)
                        do_job(emitted, qg, hh, kvq, sc)
                        emitted += 1
                # steady state with score lookahead
                pending = [emit_scores(*jobs[emitted + i]) for i in range(AHEAD)]
                for idx in range(emitted, len(jobs)):
                    qg, hh, kvq = jobs[idx]
                    sc = pending.pop(0)
                    if idx + AHEAD < len(jobs):
                        pending.append(emit_scores(*jobs[idx + AHEAD]))
                    do_job(idx, qg, hh, kvq, sc)
                flush_laggards(10 ** 9)

    nc.finalize()
    return nc


def host_prep(x, w_qkv, b_qkv, w_proj, b_proj, n_tokens=N):
    """Build per-core input maps + the host-side combine closure."""
    x = np.asarray(x, np.float32)
    w_qkv = np.asarray(w_qkv, np.float32)
    b_qkv = np.asarray(b_qkv, np.float32)
    w_proj = np.asarray(w_proj, np.float32)
    b_proj = np.asarray(b_proj, np.float32)

    xT = [np.ascontiguousarray(x[b].T).astype(ml_dtypes.bfloat16)
          for b in range(B)]  # [E, N] bf16

    in_maps = []
    for c in range(8):
        b, g = divmod(c, M_GROUPS)
        base = g * NH * 3 * HD  # row offset of this group in w_qkv (576/group)
        wq = [w_qkv[base + i * 3 * HD: base + i * 3 * HD + HD] for i in range(NH)]
        wk = [w_qkv[base + i * 3 * HD + HD: base + i * 3 * HD + 2 * HD]
              for i in range(NH)]
        wv = [w_qkv[base + i * 3 * HD + 2 * HD: base + i * 3 * HD + 3 * HD]
              for i in range(NH)]
        bqv = [b_qkv[base + i * 3 * HD: base + i * 3 * HD + HD] for i in range(NH)]
        # m-tiles: m0=[Q0;Q1], m1=[K0;K1], m2=[Q2;K2]
        wqkT = np.concatenate(
            [wq[0], wq[1], wk[0], wk[1], wq[2], wk[2]], axis=0).T  # [E, 384]
        wvT = np.concatenate(wv, axis=0).T  # [E, 192]
        bq = np.zeros((2, 128), np.float32)
        bq[0, 0:HD] = bqv[0]
        bq[0, HD:2 * HD] = bqv[1]
        bq[1, 0:HD] = bqv[2]
        # wpT[d, h, f] = w_proj[f, g*192 + h*64 + d]
        wp = w_proj[:, g * NH * HD:(g + 1) * NH * HD]  # [768, 192]
        wpT = np.ascontiguousarray(
            wp.T.reshape(NH, HD, E).transpose(1, 0, 2))  # [64, 3, 768]
        in_maps.append({
            "xT": xT[b],
            "wqkT": np.ascontiguousarray(wqkT).astype(ml_dtypes.bfloat16),
            "wvT": np.ascontiguousarray(wvT).astype(ml_dtypes.bfloat16),
            "bq": bq,
            "wpT": wpT,
            "cpow": np.full((128, 1), np.exp(0.125), np.float32),
        })

    # fold V bias through the projection into the output bias
    bv_all = np.concatenate(
        [b_qkv[h * 3 * HD + 2 * HD: (h + 1) * 3 * HD] for h in range(H)])  # [768]
    b_eff = b_proj + w_proj @ bv_all

    def combine(results):
        out = np.empty((B, n_tokens, E), np.float32)
        for b in range(B):
            acc = results[b * M_GROUPS]["out"].astype(np.float32)
            for g in range(1, M_GROUPS):
                acc = acc + results[b * M_GROUPS + g]["out"]
            out[b] = acc + b_eff
        return out

    return in_maps, combine


_NC_CACHE = {}


def kernel(x, w_qkv, b_qkv, w_proj, b_proj):
    if "nc" not in _NC_CACHE:
        _NC_CACHE["nc"] = build_nc()
    nc = _NC_CACHE["nc"]
    in_maps, combine = host_prep(x, w_qkv, b_qkv, w_proj, b_proj)
    res = run_bass_kernel_spmd(nc, in_maps, core_ids=list(range(8)))
    return combine(res.results)


if __name__ == "__main__":
    rng = np.random.default_rng(0)
    inputs = {
        "x": rng.normal(size=(B, N, E)).astype(np.float32),
        "w_qkv": (rng.normal(size=(3 * E, E)) * 0.02).astype(np.float32),
        "b_qkv": (rng.normal(size=(3 * E,)) * 0.02).astype(np.float32),
        "w_proj": (rng.normal(size=(E, E)) * 0.02).astype(np.float32),
        "b_proj": (rng.normal(size=(E,)) * 0.02).astype(np.float32),
    }
    out = kernel(**inputs)
    print("out", out.shape, out.dtype, float(np.abs(out).mean()))


# revision 23
# speedup vs baseline: 1.0074x; 1.0074x over previous
"""Multi-head attention Bass kernel for Trainium2 (8 NeuronCores).

Problem: B=2, N=4096, E=768, H=12 heads of dim 64 (nn_MultiHeadAttention).
Sharding: 2 batches x 4 head-groups (3 heads each) = 8 cores.

Per-core pipeline (fp8 DoubleRow edition):
  - QKV projection in bf16 (x and w_qkv cast to bf16 on host; 1 cyc/row).
  - Q stored as fp8e4 (hi, lo) pair: hi = e4m3(psum), lo = e4m3(psum+bq-hi),
    so hi+lo = q+bq to ~fp8^2 precision and the Q bias rides in lo.
  - K stored as fp8e4 duplicated across the two DoubleRow k-subtiles.
  - Scores: one fp8 DoubleRow matmul per kv tile: (K,K) x (Qhi,Qlo) ->
    s = (q+bq).K at 0.5 cyc/row (2x f32r, with Q at ~2^-8 precision).
  - exp: ACT Exp(scale=1/8) psum->fp8 directly, or (route split) DVE copy to
    SBUF + Pool (gpsimd) pow((e^1/8)^s) -> fp8; both bit-exact RNE.
  - PV: one fp8 DoubleRow matmul per kv PAIR (2x128 contraction):
    lhsT = V tile [128, 2, 96] (64 v-dims + ones col + 31 pad), rhs = p pair
    tile. 4x f32r throughput. Softmax denominator from the ones column.
  - Output projection in f32r against w_proj rows -> partial [N, 768].
Host: sums the 4 partials per batch and adds the (bias-folded) b_proj.

PSUM budget (8 banks): "sc" ring = 3 x 2-bank tiles shared by stage-A
psq/psv and the per-kv-pair score tiles; "pv" ring = 2 x 1-bank tiles shared
by the PV accumulators (head-major: ~one live at a time) and out-proj tiles.

Bias handling (exact algebra): K bias drops out of softmax; V bias commutes
through normalization and folds into b_proj (host); Q bias folded into Q-lo.
"""

import sys

sys.path.insert(0, "/opt/trn_rl_repo")

import numpy as np
import ml_dtypes

import concourse.bass as bass  # noqa: E402
import concourse.mybir as mybir  # noqa: E402
import concourse.tile as tile  # noqa: E402
from concourse import bacc  # noqa: E402
from concourse.bass_utils import run_bass_kernel_spmd  # noqa: E402

F32 = mybir.dt.float32
F32R = mybir.dt.float32r
BF16 = mybir.dt.bfloat16
FP8 = mybir.dt.float8e4
AF = mybir.ActivationFunctionType
ALU = mybir.AluOpType
DRMODE = mybir.MatmulPerfMode.DoubleRow

B, N, E = 2, 4096, 768
H, HD = 12, 64
NH = 3          # heads per core
M_GROUPS = 4    # head groups (tensor parallel)
VW = 96         # PV lhsT width: 64 v-dims + 1 ones + 31 pad (mult of 32)

# tuning knobs (TimelineSim-swept)
KVQ = 2         # kv tiles per score-psum tile; sc tile = [128,KVQ,512]
SC_BUFS = 3     # score-ring depth (KVQ/2 banks each)
POOL_PAT = (2, 5, 7)  # which (idx % 8) tiles take the DVE-copy + Pool pow route
AHEAD = 2       # score quads emitted ahead of the consuming exp
PVD = 4         # PV consumption delayed this many jobs behind exp emission
OST_ACT_EVERY = 4  # every Nth out-proj copy on ACT (0 = all DVE)


def build_nc(n_tokens=N, num_devices=8):
    """Build the per-core Bass module (SPMD: same program, different data)."""
    n = n_tokens
    NQG = n // 512          # q groups of 512
    NKV = n // 128          # kv blocks of 128
    KE = E // 128           # contraction tiles over E

    nc = bacc.Bacc("TRN2", target_bir_lowering=False, debug=False,
                   num_devices=num_devices)

    xT = nc.dram_tensor("xT", [E, n], BF16, kind="ExternalInput")
    wqkT = nc.dram_tensor("wqkT", [E, 3 * 128], BF16, kind="ExternalInput")
    wvT = nc.dram_tensor("wvT", [E, NH * HD], BF16, kind="ExternalInput")
    bq = nc.dram_tensor("bq", [2, 128], F32, kind="ExternalInput")
    wpT = nc.dram_tensor("wpT", [HD, NH, E], F32R, kind="ExternalInput")
    cpow = nc.dram_tensor("cpow", [128, 1], F32, kind="ExternalInput")
    out = nc.dram_tensor("out", [n, E], F32, kind="ExternalOutput")

    with tile.TileContext(nc) as tc:
        with (
            tc.tile_pool(name="perm", bufs=1) as perm,
            tc.tile_pool(name="wpool", bufs=1) as wpool,
        ):
            # Persistent SBUF tensors.
            # Q/K for heads 0 (partitions 0:64) and 1 (64:128); middle dim:
            # q_sb = (hi, lo), k_sb = duplicated K for the DR subtile pair.
            q_sb = perm.tile([128, 2, n], FP8, name="q_sb")
            k_sb = perm.tile([128, 2, n], FP8, name="k_sb")
            # head 2 on partitions 0:64
            q2_sb = perm.tile([64, 2, n], FP8, name="q2_sb")
            k2_sb = perm.tile([64, 2, n], FP8, name="k2_sb")
            # V in [kv, d] layout: per kv-block of 128 tokens,
            # 3 heads x (64 dims + ones + 31 pad).
            v_sb = perm.tile([128, NKV, NH, VW], FP8, name="v_sb")

            wqkT_sb = wpool.tile([128, KE, 3 * 128], BF16, name="wqk_sb")
            wvT_sb = wpool.tile([128, KE, NH * HD], BF16, name="wv_sb")
            wpT_sb = wpool.tile([64, NH, E], F32R, name="wp_sb")
            bq_sb = wpool.tile([128, 2], F32, name="bq_sb")
            cp_sb = wpool.tile([128, 1], F32, name="cp_sb")

            nc.sync.dma_start(wqkT_sb[:], wqkT.rearrange("(a p) c -> p a c", p=128))
            nc.sync.dma_start(wvT_sb[:], wvT.rearrange("(a p) c -> p a c", p=128))
            nc.sync.dma_start(wpT_sb[:], wpT[:])
            nc.sync.dma_start(bq_sb[:], bq.rearrange("a p -> p a"))
            nc.sync.dma_start(cp_sb[:], cpow[:])

            # ones column for the softmax-denominator trick (pad cols 65:96
            # only feed psum partitions 65:96, which are never read).
            nc.vector.memset(v_sb[:, :, :, HD:HD + 1], 1.0)

            with (
                tc.tile_pool(name="scpsum", bufs=1, space="PSUM") as scpsum,
                tc.tile_pool(name="pvpsum", bufs=1, space="PSUM") as pvpsum,
                tc.tile_pool(name="xpool", bufs=18) as xpool,
                tc.tile_pool(name="spool", bufs=3) as spool,
            ):
                # ---- Stage A: QKV projection (bf16), quantize to fp8 ----
                # psq/psv tiles ride the "sc" ring (shared with score tiles).
                def emit_stageA(ng):
                    xts = []
                    for k in range(KE):
                        xt = xpool.tile([128, 512], BF16, tag="xt",
                                        name=f"xt{ng}_{k}")
                        nc.sync.dma_start(xt[:], xT[k * 128:(k + 1) * 128,
                                                    ng * 512:(ng + 1) * 512])
                        xts.append(xt)
                    qs = slice(ng * 512, (ng + 1) * 512)
                    for m in range(3):
                        psq = scpsum.tile([128, 512], F32, tag="sc",
                                          bufs=SC_BUFS, name=f"psq{ng}_{m}")
                        for k in range(KE):
                            nc.tensor.matmul(psq[:],
                                             wqkT_sb[:, k, m * 128:(m + 1) * 128],
                                             xts[k][:], start=(k == 0),
                                             stop=(k == KE - 1))
                        if m == 0:      # Q heads 0,1
                            nc.scalar.copy(q_sb[:, 0, qs], psq[:])
                            nc.vector.scalar_tensor_tensor(
                                q_sb[:, 1, qs], psq[:], bq_sb[:, 0:1],
                                q_sb[:, 0, qs], op0=ALU.add, op1=ALU.subtract)
                        elif m == 1:    # K heads 0,1 (+ dup on Pool)
                            nc.scalar.copy(k_sb[:, 0, qs], psq[:])
                            nc.sync.dma_start(k_sb[:, 1, qs], k_sb[:, 0, qs])
                        else:           # m2 = [Q2 ; K2]
                            nc.scalar.copy(q2_sb[:, 0, qs], psq[0:64, :])
                            nc.vector.scalar_tensor_tensor(
                                q2_sb[:, 1, qs], psq[0:64, :], bq_sb[0:64, 1:2],
                                q2_sb[:, 0, qs], op0=ALU.add, op1=ALU.subtract)
                            # K2 must live on partitions 0:64 (same as Q2).
                            # DMA can't read PSUM: stage fp8 in SBUF, then a
                            # partition-shifting SBUF->SBUF DMA, then Pool dup.
                            k2st = xpool.tile([128, 512], FP8, tag="k2st",
                                              bufs=3, name=f"k2st{ng}")
                            nc.vector.tensor_copy(k2st[64:128, :],
                                                  psq[64:128, :])
                            nc.sync.dma_start(k2_sb[:, 0, qs], k2st[64:128, :])
                            nc.sync.dma_start(k2_sb[:, 1, qs], k2st[64:128, :])
                    # V projection (bf16): 2 kv-blocks per psum tile
                    for vj in range(2):
                        psv = scpsum.tile([128, 2, NH * HD], F32, tag="sc",
                                          bufs=SC_BUFS, name=f"psv{ng}_{vj}")
                        for j in range(2):
                            jj = 2 * vj + j
                            for k in range(KE):
                                nc.tensor.matmul(
                                    psv[:, j, :],
                                    xts[k][:, jj * 128:(jj + 1) * 128],
                                    wvT_sb[:, k, :], start=(k == 0),
                                    stop=(k == KE - 1))
                        kvt = ng * 4 + 2 * vj
                        nc.scalar.copy(
                            v_sb[:, kvt:kvt + 2, :, 0:HD],
                            psv.rearrange("p a (h c) -> p a h c", c=HD))

                # ---- Stage B+C: software-pipelined attention (head-major) --
                # h -> (q tile, k tile, partition base)
                HEADS = {0: (q_sb, k_sb, 0), 1: (q_sb, k_sb, 64),
                         2: (q2_sb, k2_sb, 0)}
                NQD = NKV // KVQ   # score quads per (qg, h)
                jobs = [(0, h, kvq) for kvq in range(NQD) for h in (0, 1)]
                jobs += [(0, 2, kvq) for kvq in range(NQD)]
                jobs += [(qg, h, kvq) for qg in range(1, NQG)
                         for h in range(NH) for kvq in range(NQD)]
                pvp_tiles = {}
                yn = {}

                def emit_scores(qg, h, kvq):
                    qsl = slice(qg * 512, (qg + 1) * 512)
                    qt, kt, pb = HEADS[h]
                    sc = scpsum.tile([128, KVQ, 512], F32, tag="sc",
                                     bufs=SC_BUFS, name=f"sc{qg}_{h}_{kvq}")
                    for j in range(KVQ):
                        kv = kvq * KVQ + j
                        if h == 2:
                            lhs = kt[:, :, kv * 128:(kv + 1) * 128]
                            rhs = qt[:, :, qsl]
                        else:
                            lhs = kt[pb:pb + 64, :, kv * 128:(kv + 1) * 128]
                            rhs = qt[pb:pb + 64, :, qsl]
                        nc.tensor.matmul(sc[:, j, :], lhs, rhs, start=True,
                                         stop=True, perf_mode=DRMODE)
                    return sc

                def emit_norm(qg, h):
                    pv = pvp_tiles[(qg, h)]
                    r = spool.tile([1, 512], F32, tag="r", bufs=3,
                                   name=f"r{qg}_{h}")
                    nc.vector.reciprocal(r[:], pv[HD:HD + 1, :])
                    rb = spool.tile([64, 512], F32, tag="rb", bufs=3,
                                    name=f"rb{qg}_{h}")
                    nc.gpsimd.partition_broadcast(rb[:], r[:])
                    ynt = spool.tile([64, 512], F32R, tag="yn", bufs=8,
                                     name=f"yn{qg}_{h}")
                    nc.vector.tensor_mul(ynt[:], pv[0:HD, :], rb[:])
                    yn[(qg, h)] = ynt

                def proj_thunks(qg):
                    thunks = []
                    ost_i = 0
                    for f in range(2):
                        fw = 512 if f == 0 else E - 512
                        fsl = slice(f * 512, f * 512 + fw)
                        for qb in range(4):
                            ost_i += 1
                            def blk(qg=qg, f=f, qb=qb, fw=fw, fsl=fsl,
                                    on_act=(OST_ACT_EVERY > 0
                                            and ost_i % OST_ACT_EVERY == 0)):
                                pp = pvpsum.tile([128, fw], F32, tag="pv",
                                                 bufs=2, name=f"pp{qg}_{f}_{qb}")
                                for h in range(NH):
                                    nc.tensor.matmul(
                                        pp[:],
                                        yn[(qg, h)][:, qb * 128:(qb + 1) * 128],
                                        wpT_sb[:, h, fsl],
                                        start=(h == 0), stop=(h == NH - 1))
                                ost = spool.tile([128, fw], F32, tag="ost",
                                                 bufs=6, name=f"ost{qg}_{f}_{qb}")
                                if on_act:
                                    nc.scalar.copy(ost[:], pp[:])
                                else:
                                    nc.vector.tensor_copy(ost[:], pp[:])
                                nc.sync.dma_start(
                                    out[qg * 512 + qb * 128:
                                        qg * 512 + (qb + 1) * 128, fsl], ost[:])
                            thunks.append(blk)
                    return thunks

                laggard = []   # delayed PV thunks: (emit_at_idx, fn)

                def flush_laggards(now):
                    while laggard and laggard[0][0] <= now:
                        laggard.pop(0)[1]()

                def do_job(idx, qg, hh, kvq, sc):
                    if kvq == 0:
                        pvp_tiles[(qg, hh)] = pvpsum.tile(
                            [VW, 512], F32, tag="pv", bufs=2,
                            name=f"pv{qg}_{hh}")
                    p_tile = spool.tile([128, KVQ, 512], FP8, tag="p",
                                        bufs=PVD + 4, name=f"p{qg}_{hh}_{kvq}")
                    if idx % 8 in POOL_PAT:
                        # Pool route: DVE copies scores to SBUF, gpsimd pow
                        scs = spool.tile([128, KVQ, 512], F32, tag="scs",
                                         bufs=4, name=f"scs{qg}_{hh}_{kvq}")
                        nc.vector.tensor_copy(scs[:], sc[:])
                        nc.gpsimd.tensor_tensor(
                            p_tile[:],
                            cp_sb[:, 0:1].broadcast_to([128, KVQ, 512]),
                            scs[:], op=ALU.pow)
                    else:
                        nc.scalar.activation(p_tile[:], sc[:], AF.Exp,
                                             scale=0.125)

                    def mk_pv(qg=qg, hh=hh, kvq=kvq, p_tile=p_tile):
                        def pv():
                            for t in range(KVQ // 2):
                                kv = kvq * KVQ + 2 * t
                                nc.tensor.matmul(
                                    pvp_tiles[(qg, hh)],
                                    v_sb[:, kv:kv + 2, hh, :],
                                    p_tile[:, 2 * t:2 * t + 2, :],
                                    start=(kv == 0), stop=(kv == NKV - 2),
                                    perf_mode=DRMODE)
                            if kvq == NQD - 1:
                                emit_norm(qg, hh)
                                if hh == 2:
                                    for di, blk in enumerate(proj_thunks(qg)):
                                        laggard.append((idx + PVD + 1 + di, blk))
                        return pv

                    laggard.append((idx + PVD, mk_pv()))
                    flush_laggards(idx)

                # Interleaved prologue: after stage-A chunk ng, run the two
                # first-head jobs (qg0, h0, kvq = 2ng, 2ng+1) whose K/V were
                # just produced. Keeps ACT/Pool fed during the projections.
                emitted = 0
                for ng in range(NQG):
                    emit_stageA(ng)
                    for _ in range(2 * NQD // NQG):
                        qg, hh, kvq = jobs[emitted]
                        sc = emit_scores(qg, hh, kvq)
                        do_job(emitted, qg, hh, kvq, sc)
                        emitted += 1
                # steady state with score lookahead
                pending = [emit_scores(*jobs[emitted + i]) for i in range(AHEAD)]
                for idx in range(emitted, len(jobs)):
                    qg, hh, kvq = jobs[idx]
                    sc = pending.pop(0)
                    if idx + AHEAD < len(jobs):
                        pending.append(emit_scores(*jobs[idx + AHEAD]))
                    do_job(idx, qg, hh, kvq, sc)
                flush_laggards(10 ** 9)

    nc.finalize()
    return nc


def host_prep(x, w_qkv, b_qkv, w_proj, b_proj, n_tokens=N):
    """Build per-core input maps + the host-side combine closure."""
    x = np.asarray(x, np.float32)
    w_qkv = np.asarray(w_qkv, np.float32)
    b_qkv = np.asarray(b_qkv, np.float32)
    w_proj = np.asarray(w_proj, np.float32)
    b_proj = np.asarray(b_proj, np.float32)

    xT = [np.ascontiguousarray(x[b].T).astype(ml_dtypes.bfloat16)
          for b in range(B)]  # [E, N] bf16

    in_maps = []
    for c in range(8):
        b, g = divmod(c, M_GROUPS)
        base = g * NH * 3 * HD  # row offset of this group in w_qkv (576/group)
        wq = [w_qkv[base + i * 3 * HD: base + i * 3 * HD + HD] for i in range(NH)]
        wk = [w_qkv[base + i * 3 * HD + HD: base + i * 3 * HD + 2 * HD]
              for i in range(NH)]
        wv = [w_qkv[base + i * 3 * HD + 2 * HD: base + i * 3 * HD + 3 * HD]
              for i in range(NH)]
        bqv = [b_qkv[base + i * 3 * HD: base + i * 3 * HD + HD] for i in range(NH)]
        # m-tiles: m0=[Q0;Q1], m1=[K0;K1], m2=[Q2;K2]
        wqkT = np.concatenate(
            [wq[0], wq[1], wk[0], wk[1], wq[2], wk[2]], axis=0).T  # [E, 384]
        wvT = np.concatenate(wv, axis=0).T  # [E, 192]
        bq = np.zeros((2, 128), np.float32)
        bq[0, 0:HD] = bqv[0]
        bq[0, HD:2 * HD] = bqv[1]
        bq[1, 0:HD] = bqv[2]
        # wpT[d, h, f] = w_proj[f, g*192 + h*64 + d]
        wp = w_proj[:, g * NH * HD:(g + 1) * NH * HD]  # [768, 192]
        wpT = np.ascontiguousarray(
            wp.T.reshape(NH, HD, E).transpose(1, 0, 2))  # [64, 3, 768]
        in_maps.append({
            "xT": xT[b],
            "wqkT": np.ascontiguousarray(wqkT).astype(ml_dtypes.bfloat16),
            "wvT": np.ascontiguousarray(wvT).astype(ml_dtypes.bfloat16),
            "bq": bq,
            "wpT": wpT,
            "cpow": np.full((128, 1), np.exp(0.125), np.float32),
        })

    # fold V bias through the projection into the output bias
    bv_all = np.concatenate(
        [b_qkv[h * 3 * HD + 2 * HD: (h + 1) * 3 * HD] for h in range(H)])  # [768]
    b_eff = b_proj + w_proj @ bv_all

    def combine(results):
        out = np.empty((B, n_tokens, E), np.float32)
        for b in range(B):
            acc = results[b * M_GROUPS]["out"].astype(np.float32)
            for g in range(1, M_GROUPS):
                acc = acc + results[b * M_GROUPS + g]["out"]
            out[b] = acc + b_eff
        return out

    return in_maps, combine


_NC_CACHE = {}


def kernel(x, w_qkv, b_qkv, w_proj, b_proj):
    if "nc" not in _NC_CACHE:
        _NC_CACHE["nc"] = build_nc()
    nc = _NC_CACHE["nc"]
    in_maps, combine = host_prep(x, w_qkv, b_qkv, w_proj, b_proj)
    res = run_bass_kernel_spmd(nc, in_maps, core_ids=list(range(8)))
    return combine(res.results)


if __name__ == "__main__":
    rng = np.random.default_rng(0)
    inputs = {
        "x": rng.normal(size=(B, N, E)).astype(np.float32),
        "w_qkv": (rng.normal(size=(3 * E, E)) * 0.02).astype(np.float32),
        "b_qkv": (rng.normal(size=(3 * E,)) * 0.02).astype(np.float32),
        "w_proj": (rng.normal(size=(E, E)) * 0.02).astype(np.float32),
        "b_proj": (rng.normal(size=(E,)) * 0.02).astype(np.float32),
    }
    out = kernel(**inputs)
    print("out", out.shape, out.dtype, float(np.abs(out).mean()))


# revision 27
# speedup vs baseline: 1.0090x; 1.0015x over previous
"""Multi-head attention Bass kernel for Trainium2 (8 NeuronCores).

Problem: B=2, N=4096, E=768, H=12 heads of dim 64 (nn_MultiHeadAttention).
Sharding: 2 batches x 4 head-groups (3 heads each) = 8 cores.

Per-core pipeline (fp8 DoubleRow edition):
  - QKV projection in bf16 (x and w_qkv cast to bf16 on host; 1 cyc/row).
  - Q stored as fp8e4 (hi, lo) pair: hi = e4m3(psum), lo = e4m3(psum+bq-hi),
    so hi+lo = q+bq to ~fp8^2 precision and the Q bias rides in lo.
  - K stored as fp8e4 duplicated across the two DoubleRow k-subtiles.
  - Scores: one fp8 DoubleRow matmul per kv tile: (K,K) x (Qhi,Qlo) ->
    s = (q+bq).K at 0.5 cyc/row (2x f32r, with Q at ~2^-8 precision).
  - exp: ACT Exp(scale=1/8) psum->fp8 directly, or (route split) DVE copy to
    SBUF + Pool (gpsimd) pow((e^1/8)^s) -> fp8; both bit-exact RNE.
  - PV: one fp8 DoubleRow matmul per kv PAIR (2x128 contraction):
    lhsT = V tile [128, 2, 96] (64 v-dims + ones col + 31 pad), rhs = p pair
    tile. 4x f32r throughput. Softmax denominator from the ones column.
  - Output projection in f32r against w_proj rows -> partial [N, 768].
Host: sums the 4 partials per batch and adds the (bias-folded) b_proj.

PSUM budget (8 banks): "sc" ring = 3 x 2-bank tiles shared by stage-A
psq/psv and the per-kv-pair score tiles; "pv" ring = 2 x 1-bank tiles shared
by the PV accumulators (head-major: ~one live at a time) and out-proj tiles.

Bias handling (exact algebra): K bias drops out of softmax; V bias commutes
through normalization and folds into b_proj (host); Q bias folded into Q-lo.
"""

import sys

sys.path.insert(0, "/opt/trn_rl_repo")

import numpy as np
import ml_dtypes

import concourse.bass as bass  # noqa: E402
import concourse.mybir as mybir  # noqa: E402
import concourse.tile as tile  # noqa: E402
from concourse import bacc  # noqa: E402
from concourse.bass_utils import run_bass_kernel_spmd  # noqa: E402

F32 = mybir.dt.float32
F32R = mybir.dt.float32r
BF16 = mybir.dt.bfloat16
FP8 = mybir.dt.float8e4
AF = mybir.ActivationFunctionType
ALU = mybir.AluOpType
DRMODE = mybir.MatmulPerfMode.DoubleRow

B, N, E = 2, 4096, 768
H, HD = 12, 64
NH = 3          # heads per core
M_GROUPS = 4    # head groups (tensor parallel)
VW = 96         # PV lhsT width: 64 v-dims + 1 ones + 31 pad (mult of 32)

# tuning knobs (TimelineSim-swept)
KVQ = 2         # kv tiles per score-psum tile; sc tile = [128,KVQ,512]
SC_BUFS = 3     # score-ring depth (KVQ/2 banks each)
POOL_PAT = (2, 5, 7)  # which (idx % 8) tiles take the DVE-copy + Pool pow route
AHEAD = 2       # score quads emitted ahead of the consuming exp
PVD = 4         # PV consumption delayed this many jobs behind exp emission
OST_ACT_EVERY = 4  # every Nth out-proj copy on ACT (0 = all DVE)


def build_nc(n_tokens=N, num_devices=8):
    """Build the per-core Bass module (SPMD: same program, different data)."""
    n = n_tokens
    NQG = n // 512          # q groups of 512
    NKV = n // 128          # kv blocks of 128
    KE = E // 128           # contraction tiles over E

    nc = bacc.Bacc("TRN2", target_bir_lowering=False, debug=False,
                   num_devices=num_devices)

    xT = nc.dram_tensor("xT", [E, n], BF16, kind="ExternalInput")
    wqkT = nc.dram_tensor("wqkT", [E, 3 * 128], BF16, kind="ExternalInput")
    wvT = nc.dram_tensor("wvT", [E, NH * HD], BF16, kind="ExternalInput")
    bq = nc.dram_tensor("bq", [2, 128], F32, kind="ExternalInput")
    wpT = nc.dram_tensor("wpT", [HD, NH, E], F32R, kind="ExternalInput")
    cpow = nc.dram_tensor("cpow", [128, 1], F32, kind="ExternalInput")
    out = nc.dram_tensor("out", [n, E], F32, kind="ExternalOutput")

    with tile.TileContext(nc) as tc:
        with (
            tc.tile_pool(name="perm", bufs=1) as perm,
            tc.tile_pool(name="wpool", bufs=1) as wpool,
        ):
            # Persistent SBUF tensors.
            # Q/K for heads 0 (partitions 0:64) and 1 (64:128); middle dim:
            # q_sb = (hi, lo), k_sb = duplicated K for the DR subtile pair.
            q_sb = perm.tile([128, 2, n], FP8, name="q_sb")
            k_sb = perm.tile([128, 2, n], FP8, name="k_sb")
            # head 2 on partitions 0:64
            q2_sb = perm.tile([64, 2, n], FP8, name="q2_sb")
            k2_sb = perm.tile([64, 2, n], FP8, name="k2_sb")
            # V in [kv, d] layout: per kv-block of 128 tokens,
            # 3 heads x (64 dims + ones + 31 pad).
            v_sb = perm.tile([128, NKV, NH, VW], FP8, name="v_sb")

            wqkT_sb = wpool.tile([128, KE, 3 * 128], BF16, name="wqk_sb")
            wvT_sb = wpool.tile([128, KE, NH * HD], BF16, name="wv_sb")
            wpT_sb = wpool.tile([64, NH, E], F32R, name="wp_sb")
            bq_sb = wpool.tile([128, 2], F32, name="bq_sb")
            cp_sb = wpool.tile([128, 1], F32, name="cp_sb")

            nc.sync.dma_start(wqkT_sb[:], wqkT.rearrange("(a p) c -> p a c", p=128))
            nc.sync.dma_start(wvT_sb[:], wvT.rearrange("(a p) c -> p a c", p=128))
            nc.sync.dma_start(wpT_sb[:], wpT[:])
            nc.sync.dma_start(bq_sb[:], bq.rearrange("a p -> p a"))
            nc.sync.dma_start(cp_sb[:], cpow[:])

            # ones column for the softmax-denominator trick (pad cols 65:96
            # only feed psum partitions 65:96, which are never read).
            nc.vector.memset(v_sb[:, :, :, HD:HD + 1], 1.0)

            with (
                tc.tile_pool(name="scpsum", bufs=1, space="PSUM") as scpsum,
                tc.tile_pool(name="pvpsum", bufs=1, space="PSUM") as pvpsum,
                tc.tile_pool(name="xpool", bufs=18) as xpool,
                tc.tile_pool(name="spool", bufs=3) as spool,
            ):
                # PE p-state warmup: tiny matmuls on garbage SBUF into a
                # dummy psum tile during the initial DMA wait; psum is never
                # read. 3us of continuous PE activity reaches full clock.
                scratch = wpool.tile([64, 256], FP8, name="warm_src")
                nc.gpsimd.memset(scratch[:], 0.0)
                warm = pvpsum.tile([128, 64], F32, tag="pv", bufs=2,
                                   name="warmup")
                for wi in range(40):
                    nc.tensor.matmul(warm[:], scratch[:, 0:128],
                                     scratch[:, 0:64],
                                     start=(wi == 0), stop=(wi == 39))

                # ---- Stage A: QKV projection (bf16), quantize to fp8 ----
                # psq/psv tiles ride the "sc" ring (shared with score tiles).
                def emit_stageA(ng):
                    xts = []
                    for k in range(KE):
                        xt = xpool.tile([128, 512], BF16, tag="xt",
                                        name=f"xt{ng}_{k}")
                        nc.sync.dma_start(xt[:], xT[k * 128:(k + 1) * 128,
                                                    ng * 512:(ng + 1) * 512])
                        xts.append(xt)
                    qs = slice(ng * 512, (ng + 1) * 512)
                    for m in range(3):
                        psq = scpsum.tile([128, 512], F32, tag="sc",
                                          bufs=SC_BUFS, name=f"psq{ng}_{m}")
                        for k in range(KE):
                            nc.tensor.matmul(psq[:],
                                             wqkT_sb[:, k, m * 128:(m + 1) * 128],
                                             xts[k][:], start=(k == 0),
                                             stop=(k == KE - 1))
                        if m == 0:      # Q heads 0,1
                            nc.scalar.copy(q_sb[:, 0, qs], psq[:])
                            nc.vector.scalar_tensor_tensor(
                                q_sb[:, 1, qs], psq[:], bq_sb[:, 0:1],
                                q_sb[:, 0, qs], op0=ALU.add, op1=ALU.subtract)
                        elif m == 1:    # K heads 0,1 (+ dup on Pool)
                            nc.scalar.copy(k_sb[:, 0, qs], psq[:])
                            nc.sync.dma_start(k_sb[:, 1, qs], k_sb[:, 0, qs])
                        else:           # m2 = [Q2 ; K2]
                            nc.scalar.copy(q2_sb[:, 0, qs], psq[0:64, :])
                            nc.vector.scalar_tensor_tensor(
                                q2_sb[:, 1, qs], psq[0:64, :], bq_sb[0:64, 1:2],
                                q2_sb[:, 0, qs], op0=ALU.add, op1=ALU.subtract)
                            # K2 must live on partitions 0:64 (same as Q2).
                            # DMA can't read PSUM: stage fp8 in SBUF, then a
                            # partition-shifting SBUF->SBUF DMA, then Pool dup.
                            k2st = xpool.tile([128, 512], FP8, tag="k2st",
                                              bufs=3, name=f"k2st{ng}")
                            nc.vector.tensor_copy(k2st[64:128, :],
                                                  psq[64:128, :])
                            nc.sync.dma_start(k2_sb[:, 0, qs], k2st[64:128, :])
                            nc.sync.dma_start(k2_sb[:, 1, qs], k2st[64:128, :])
                    # V projection (bf16): 2 kv-blocks per psum tile
                    for vj in range(2):
                        psv = scpsum.tile([128, 2, NH * HD], F32, tag="sc",
                                          bufs=SC_BUFS, name=f"psv{ng}_{vj}")
                        for j in range(2):
                            jj = 2 * vj + j
                            for k in range(KE):
                                nc.tensor.matmul(
                                    psv[:, j, :],
                                    xts[k][:, jj * 128:(jj + 1) * 128],
                                    wvT_sb[:, k, :], start=(k == 0),
                                    stop=(k == KE - 1))
                        kvt = ng * 4 + 2 * vj
                        nc.scalar.copy(
                            v_sb[:, kvt:kvt + 2, :, 0:HD],
                            psv.rearrange("p a (h c) -> p a h c", c=HD))

                # ---- Stage B+C: software-pipelined attention (head-major) --
                # h -> (q tile, k tile, partition base)
                HEADS = {0: (q_sb, k_sb, 0), 1: (q_sb, k_sb, 64),
                         2: (q2_sb, k2_sb, 0)}
                NQD = NKV // KVQ   # score quads per (qg, h)
                jobs = [(0, h, kvq) for kvq in range(NQD) for h in (0, 1)]
                jobs += [(0, 2, kvq) for kvq in range(NQD)]
                jobs += [(qg, h, kvq) for qg in range(1, NQG)
                         for h in range(NH) for kvq in range(NQD)]
                NJOBS = len(jobs)
                pvp_tiles = {}
                yn = {}

                def emit_scores(qg, h, kvq):
                    qsl = slice(qg * 512, (qg + 1) * 512)
                    qt, kt, pb = HEADS[h]
                    sc = scpsum.tile([128, KVQ, 512], F32, tag="sc",
                                     bufs=SC_BUFS, name=f"sc{qg}_{h}_{kvq}")
                    for j in range(KVQ):
                        kv = kvq * KVQ + j
                        if h == 2:
                            lhs = kt[:, :, kv * 128:(kv + 1) * 128]
                            rhs = qt[:, :, qsl]
                        else:
                            lhs = kt[pb:pb + 64, :, kv * 128:(kv + 1) * 128]
                            rhs = qt[pb:pb + 64, :, qsl]
                        nc.tensor.matmul(sc[:, j, :], lhs, rhs, start=True,
                                         stop=True, perf_mode=DRMODE)
                    return sc

                def emit_norm(qg, h):
                    pv = pvp_tiles[(qg, h)]
                    r = spool.tile([1, 512], F32, tag="r", bufs=3,
                                   name=f"r{qg}_{h}")
                    nc.vector.reciprocal(r[:], pv[HD:HD + 1, :])
                    rb = spool.tile([64, 512], F32, tag="rb", bufs=3,
                                    name=f"rb{qg}_{h}")
                    nc.gpsimd.partition_broadcast(rb[:], r[:])
                    ynt = spool.tile([64, 512], F32R, tag="yn", bufs=8,
                                     name=f"yn{qg}_{h}")
                    nc.vector.tensor_mul(ynt[:], pv[0:HD, :], rb[:])
                    yn[(qg, h)] = ynt

                def proj_thunks(qg):
                    thunks = []
                    ost_i = 0
                    for f in range(2):
                        fw = 512 if f == 0 else E - 512
                        fsl = slice(f * 512, f * 512 + fw)
                        for qb in range(4):
                            ost_i += 1
                            def blk(qg=qg, f=f, qb=qb, fw=fw, fsl=fsl,
                                    on_act=(OST_ACT_EVERY > 0
                                            and ost_i % OST_ACT_EVERY == 0)):
                                pp = pvpsum.tile([128, fw], F32, tag="pv",
                                                 bufs=2, name=f"pp{qg}_{f}_{qb}")
                                for h in range(NH):
                                    nc.tensor.matmul(
                                        pp[:],
                                        yn[(qg, h)][:, qb * 128:(qb + 1) * 128],
                                        wpT_sb[:, h, fsl],
                                        start=(h == 0), stop=(h == NH - 1))
                                ost = spool.tile([128, fw], F32, tag="ost",
                                                 bufs=6, name=f"ost{qg}_{f}_{qb}")
                                if on_act:
                                    nc.scalar.copy(ost[:], pp[:])
                                else:
                                    nc.vector.tensor_copy(ost[:], pp[:])
                                nc.sync.dma_start(
                                    out[qg * 512 + qb * 128:
                                        qg * 512 + (qb + 1) * 128, fsl], ost[:])
                            thunks.append(blk)
                    return thunks

                laggard = []   # delayed PV thunks: (emit_at_idx, fn)

                def flush_laggards(now):
                    while laggard and laggard[0][0] <= now:
                        laggard.pop(0)[1]()

                def do_job(idx, qg, hh, kvq, sc):
                    if kvq == 0:
                        pvp_tiles[(qg, hh)] = pvpsum.tile(
                            [VW, 512], F32, tag="pv", bufs=2,
                            name=f"pv{qg}_{hh}")
                    p_tile = spool.tile([128, KVQ, 512], FP8, tag="p",
                                        bufs=PVD + 4, name=f"p{qg}_{hh}_{kvq}")
                    if idx % 8 in POOL_PAT:
                        # Pool route: DVE copies scores to SBUF, gpsimd pow
                        scs = spool.tile([128, KVQ, 512], F32, tag="scs",
                                         bufs=4, name=f"scs{qg}_{hh}_{kvq}")
                        nc.vector.tensor_copy(scs[:], sc[:])
                        nc.gpsimd.tensor_tensor(
                            p_tile[:],
                            cp_sb[:, 0:1].broadcast_to([128, KVQ, 512]),
                            scs[:], op=ALU.pow)
                    else:
                        nc.scalar.activation(p_tile[:], sc[:], AF.Exp,
                                             scale=0.125)

                    def mk_pv(qg=qg, hh=hh, kvq=kvq, p_tile=p_tile):
                        def pv():
                            for t in range(KVQ // 2):
                                kv = kvq * KVQ + 2 * t
                                nc.tensor.matmul(
                                    pvp_tiles[(qg, hh)],
                                    v_sb[:, kv:kv + 2, hh, :],
                                    p_tile[:, 2 * t:2 * t + 2, :],
                                    start=(kv == 0), stop=(kv == NKV - 2),
                                    perf_mode=DRMODE)
                            if kvq == NQD - 1:
                                emit_norm(qg, hh)
                                if hh == 2:
                                    for di, blk in enumerate(proj_thunks(qg)):
                                        laggard.append(
                                            (min(idx + PVD + 1 + di, NJOBS - 1), blk))
                        return pv

                    laggard.append((min(idx + PVD, NJOBS - 1), mk_pv()))
                    flush_laggards(idx)

                # Interleaved prologue: after stage-A chunk ng, run the two
                # first-head jobs (qg0, h0, kvq = 2ng, 2ng+1) whose K/V were
                # just produced. Keeps ACT/Pool fed during the projections.
                emitted = 0
                for ng in range(NQG):
                    emit_stageA(ng)
                    for _ in range(2 * NQD // NQG):
                        qg, hh, kvq = jobs[emitted]
                        sc = emit_scores(qg, hh, kvq)
                        do_job(emitted, qg, hh, kvq, sc)
                        emitted += 1
                # steady state with score lookahead
                pending = [emit_scores(*jobs[emitted + i]) for i in range(AHEAD)]
                for idx in range(emitted, len(jobs)):
                    qg, hh, kvq = jobs[idx]
                    sc = pending.pop(0)
                    if idx + AHEAD < len(jobs):
                        pending.append(emit_scores(*jobs[idx + AHEAD]))
                    do_job(idx, qg, hh, kvq, sc)
                flush_laggards(10 ** 9)

    nc.finalize()
    return nc


def host_prep(x, w_qkv, b_qkv, w_proj, b_proj, n_tokens=N):
    """Build per-core input maps + the host-side combine closure."""
    x = np.asarray(x, np.float32)
    w_qkv = np.asarray(w_qkv, np.float32)
    b_qkv = np.asarray(b_qkv, np.float32)
    w_proj = np.asarray(w_proj, np.float32)
    b_proj = np.asarray(b_proj, np.float32)

    xT = [np.ascontiguousarray(x[b].T).astype(ml_dtypes.bfloat16)
          for b in range(B)]  # [E, N] bf16

    in_maps = []
    for c in range(8):
        b, g = divmod(c, M_GROUPS)
        base = g * NH * 3 * HD  # row offset of this group in w_qkv (576/group)
        wq = [w_qkv[base + i * 3 * HD: base + i * 3 * HD + HD] for i in range(NH)]
        wk = [w_qkv[base + i * 3 * HD + HD: base + i * 3 * HD + 2 * HD]
              for i in range(NH)]
        wv = [w_qkv[base + i * 3 * HD + 2 * HD: base + i * 3 * HD + 3 * HD]
              for i in range(NH)]
        bqv = [b_qkv[base + i * 3 * HD: base + i * 3 * HD + HD] for i in range(NH)]
        # m-tiles: m0=[Q0;Q1], m1=[K0;K1], m2=[Q2;K2]
        wqkT = np.concatenate(
            [wq[0], wq[1], wk[0], wk[1], wq[2], wk[2]], axis=0).T  # [E, 384]
        wvT = np.concatenate(wv, axis=0).T  # [E, 192]
        bq = np.zeros((2, 128), np.float32)
        bq[0, 0:HD] = bqv[0]
        bq[0, HD:2 * HD] = bqv[1]
        bq[1, 0:HD] = bqv[2]
        # wpT[d, h, f] = w_proj[f, g*192 + h*64 + d]
        wp = w_proj[:, g * NH * HD:(g + 1) * NH * HD]  # [768, 192]
        wpT = np.ascontiguousarray(
            wp.T.reshape(NH, HD, E).transpose(1, 0, 2))  # [64, 3, 768]
        in_maps.append({
            "xT": xT[b],
            "wqkT": np.ascontiguousarray(wqkT).astype(ml_dtypes.bfloat16),
            "wvT": np.ascontiguousarray(wvT).astype(ml_dtypes.bfloat16),
            "bq": bq,
            "wpT": wpT,
            "cpow": np.full((128, 1), np.exp(0.125), np.float32),
        })

    # fold V bias through the projection into the output bias
    bv_all = np.concatenate(
        [b_qkv[h * 3 * HD + 2 * HD: (h + 1) * 3 * HD] for h in range(H)])  # [768]
    b_eff = b_proj + w_proj @ bv_all

    def combine(results):
        out = np.empty((B, n_tokens, E), np.float32)
        for b in range(B):
            acc = results[b * M_GROUPS]["out"].astype(np.float32)
            for g in range(1, M_GROUPS):
                acc = acc + results[b * M_GROUPS + g]["out"]
            out[b] = acc + b_eff
        return out

    return in_maps, combine


_NC_CACHE = {}


def kernel(x, w_qkv, b_qkv, w_proj, b_proj):
    if "nc" not in _NC_CACHE:
        _NC_CACHE["nc"] = build_nc()
    nc = _NC_CACHE["nc"]
    in_maps, combine = host_prep(x, w_qkv, b_qkv, w_proj, b_proj)
    res = run_bass_kernel_spmd(nc, in_maps, core_ids=list(range(8)))
    return combine(res.results)


if __name__ == "__main__":
    rng = np.random.default_rng(0)
    inputs = {
        "x": rng.normal(size=(B, N, E)).astype(np.float32),
        "w_qkv": (rng.normal(size=(3 * E, E)) * 0.02).astype(np.float32),
        "b_qkv": (rng.normal(size=(3 * E,)) * 0.02).astype(np.float32),
        "w_proj": (rng.normal(size=(E, E)) * 0.02).astype(np.float32),
        "b_proj": (rng.normal(size=(E,)) * 0.02).astype(np.float32),
    }
    out = kernel(**inputs)
    print("out", out.shape, out.dtype, float(np.abs(out).mean()))
